# revision 59
# baseline (speedup 1.0000x reference)
"""Multi-head attention Trainium2 kernel (Bass/Tile), 8-core SPMD.

Problem: B=2, S=2048, D=1024, H=16 heads of d=64.
Sharding: core c -> batch c//4, 4 heads starting at 4*(c%4).
Each core computes its heads' Q/K/V projections, attention, and the
partial output projection (transposed); host sums the 4 bf16 partials
per batch and adds bo.

Device-side layout notes:
  - Projection activations live transposed ([feature, seq]) so every
    matmul contraction runs over the partition dim.
  - Scores are computed transposed (S^T[sk, sq]) so that P^T = exp(S^T)
    feeds the P@V matmul directly as the STATIONARY operand.
  - P@V runs with V as the MOVING operand: out tiles are [128 q, 65]
    per (head, q-subtile) with pt [128 k, 128 q] stationary. The PE is
    charged per moving column, so each k-chunk costs 65 cycles instead
    of the 512 a [65, 512]-out layout pays (the old layout used only
    65 of 128 output partitions). V carries an extra ones column so
    the same matmuls emit softmax row-sums for free (col 64).
  - U accumulates in SBUF as [q-part, qtile, head, 65] fp32; row-sums
    land per-partition, so normalization is a plain per-partition
    tensor_scalar multiply on DVE (no partition broadcast needed).
  - The normalized attention is built as aTpre [q, d-of-pair] bf16 and
    flipped to aT [d, q] with the DMA-engine XBAR transpose (16x128
    tiles, off all compute engines); the output projection then runs
    unchanged with aT as the moving operand.
  - x/weights/aT/pt/v/output are bf16 (same PE rate as fp32r at 256+
    moving cols, required for full rate at 65 cols); scores stay fp32.

Schedule: the attention inner loop is ACT-bound (exp of a [128,1024]
score tile is ~1040ns vs ~645ns of score+PV matmuls), so all
projection and epilogue work is chopped into ~430ns chunks and emitted
between the exp and PV of each iteration via a credit-paced filler
queue (labels + need() keep emission order consistent with data flow).
Q proj for block j rides block j-1's iterations; K/V proj for key
block sb+1 ride sb's; normalize (DVE) + transpose (DMA) + output
projection for query block isq ride the last key block's later
iterations. PSUM: 2 banks filler ring + 4 banks score tiles + 2 banks
U accumulators.
"""

import numpy as np

import concourse.bass as bass
import concourse.mybir as mybir
import concourse.tile as tile
from concourse import bacc
from concourse.bass_utils import run_bass_kernel_spmd

F32 = mybir.dt.float32
F32R = mybir.dt.float32r
BF16 = mybir.dt.bfloat16
AF = mybir.ActivationFunctionType

B, S, D = 2, 2048, 1024
H, DH = 16, 64
NCORES = 8
HL = H // (NCORES // B)       # 4 heads per core
DL = HL * DH                  # 256 local projection dims
PAIRS = HL // 2               # 2 head pairs (packed into 128 partitions)
NKT = D // 128                # 8 contraction tiles for projections
SB = 512                      # seq block (matmul moving-dim chunk)
NSB = S // SB                 # 4
NSKT = S // 128               # 16 key-seq tiles
NQT = S // 128                # 16 query subtiles of 128
SCALE = 0.125                 # 1/sqrt(64)
# f32 whose bit pattern is two bf16 1.0s — used to memset a bf16 tile
# to all-ones through its f32 bitcast view (strided/bf16 memsets fail
# walrus ISA checks).
ONES2_BF16 = 1.0019378662109375

LAST_EXEC_NS = None
_TRACE = False
_TRACE_KW = {}


def _bcast_part(ap, parts):
    """View `ap` with the partition dim replaced by a step-0 broadcast."""
    return bass.AP(tensor=ap.tensor, offset=ap.offset, ap=[[0, parts]] + list(ap.ap[1:]))


def _emit(tc, nc, t):
    import contextlib

    ctx = contextlib.ExitStack()
    with ctx:
        const = ctx.enter_context(tc.tile_pool(name="const", bufs=1))
        persist = ctx.enter_context(tc.tile_pool(name="persist", bufs=1))
        # x chunks are one DMA per (tensor, s-block): [128, 8, 512] bf16, 8KB
        # per partition; 6 bufs = one in use + prefetch headroom.
        xin = ctx.enter_context(tc.tile_pool(name="xin", bufs=6))
        ptp = ctx.enter_context(tc.tile_pool(name="ptp", bufs=4))
        outp = ctx.enter_context(tc.tile_pool(name="outp", bufs=4))
        misc = ctx.enter_context(tc.tile_pool(name="misc", bufs=4))

        # ---------- constants & weights ----------
        # Weight DMAs are split in halves and emitted interleaved with the x
        # chunks that pair with them (inside the projection loops below), so
        # the first matmul starts ~2us in instead of waiting ~14us of
        # serialized weight transfers. Weights and x are bf16: same PE rate
        # as fp32r in this regime but half the HBM traffic.
        wq_sb = const.tile([128, NKT, DL], BF16)
        wk_sb = const.tile([128, NKT, DL], BF16)
        wv_sb = const.tile([128, NKT, DL], BF16)
        wo_sb = const.tile([128, 2, D], BF16)
        bq_sb = const.tile([128, 2], F32)
        bk_sb = const.tile([128, 2], F32)
        bvb = const.tile([128, DL], F32)

        def load_biases():
            # bq/bk ride the Pool SWDGE queue (tiny transfers, off the
            # HWDGE/DMA critical path of the prologue); the broadcast bvb
            # stays on HWDGE and is only needed by the V evacuations.
            nc.gpsimd.dma_start(out=bq_sb, in_=t["bq"].rearrange("(t p) -> p t", p=128))
            nc.gpsimd.dma_start(out=bk_sb, in_=t["bk"].rearrange("(t p) -> p t", p=128))
            nc.sync.dma_start(out=bvb,
                              in_=_bcast_part(t["bv"].rearrange("(o d) -> o d", o=1), 128))

        # ---------- persistent activations ----------
        qT = persist.tile([128, PAIRS, S], F32R)   # [dh-in-pair, pair, s]
        kT = persist.tile([128, PAIRS, S], F32R)
        v_flat = persist.tile([128, NSKT, HL * (DH + 1)], BF16)
        # [sk-in-tile, skt, head, d|1]
        v_sb = v_flat.rearrange("p t (h d) -> p t h d", h=HL)
        aT = persist.tile([128, PAIRS, S], BF16)   # normalized attention, transposed
        aTpre = persist.tile([128, PAIRS, NQT, 128], BF16)  # [q, pair, qt, hi*64+d]
        uacc = persist.tile([128, NQT, HL, DH + 1], F32)    # U accumulator (SBUF)
        # Whole-tile memset through the f32 bitcast view: value is the bit
        # pattern of two bf16 1.0s. The V evacuations overwrite everything
        # except the ones column.
        nc.gpsimd.memset(v_flat.bitcast(F32), ONES2_BF16)
        ones_row = persist.tile([1, 128], F32R)
        nc.vector.memset(ones_row.bitcast(F32), 1.0)
        # Warm-up exp so the ACT table set loads at t~0 instead of on the
        # first real exp's critical path.
        warm = persist.tile([1, 1], F32)
        nc.scalar.activation(warm, ones_row.bitcast(F32)[0:1, 0:1], AF.Exp)

        # ---------- shared psum pools ----------
        # 8 banks total: pp 2 (projection/outproj ring), p_big 4 (score
        # tiles), p_ut 2 (U accumulators).
        pp = ctx.enter_context(tc.tile_pool(name="pp", bufs=2, space="PSUM"))
        p_big = ctx.enter_context(tc.tile_pool(name="p_big", bufs=2, space="PSUM"))
        p_ut = ctx.enter_context(tc.tile_pool(name="p_ut", bufs=2, space="PSUM"))

        # out DRAM viewed [mg, p, mi, s]: row (mg*2+mi)*128 + p, col s
        out4 = t["outF"].rearrange("(g m p) s -> g p m s", g=4, m=2)

        def load_w_half(w_sb, name, half):
            nc.sync.dma_start(
                out=w_sb[:, half * 4:(half + 1) * 4, :],
                in_=t[name][half * 512:(half + 1) * 512, :]
                .rearrange("(c p) d -> p c d", p=128))

        def load_x(name, sb, parts=(NKT,)):
            # one DMA per (tensor, s-block); the prologue passes graded part
            # sizes (small first) so the first matmuls start ~2us in.
            xt = xin.tile([128, NKT, SB], BF16, name=f"x_{name}_{sb}", tag="x")
            k0 = 0
            for kpp in parts:
                nc.sync.dma_start(
                    out=xt[:, k0:k0 + kpp, :],
                    in_=t[name][k0 * 128:(k0 + kpp) * 128,
                                sb * SB:(sb + 1) * SB]
                    .rearrange("(c p) s -> p c s", p=128))
                k0 += kpp
            return xt

        # ---------- filler machinery ----------
        # PE work chunks (~400-900ns each) queued up and emitted between the
        # exp and PV of each attention iteration: the attention inner loop is
        # otherwise ACT-bound (exp ~1040ns vs ~645ns of PE work per
        # iteration), so the projections/epilogue ride in the PE bubbles for
        # free. Chunks carry labels; attention() declares the chunks it
        # depends on via need() (emission-order = program order, so a
        # consumer emitted before its producer would silently read stale
        # data).
        filler = []
        done, pending = set(), set()
        pace = {"rate": 0.0, "credit": 0.0, "iters": 1, "itc": 0}

        def _pe_len():
            return sum(1 for it in filler if it[2])

        def pop_one():
            label, fn, is_pe, _gate = filler.pop(0)
            fn()
            if label:
                done.add(label)
            return is_pe

        def pump():
            # credit pacing: spread the PE-bearing chunks evenly over the
            # remaining attention iterations of this key block (an unfilled
            # iteration stalls the PE ~400ns). Chunks with no PE content
            # (normalize/transpose/evacuate) ride along for free. A chunk
            # with a gate > current iteration count blocks the queue head:
            # used to hold the output projection back until its normalize ->
            # transpose chain (emitted the same boundary) has had time to
            # land.
            pace["itc"] += 1
            pace["credit"] += pace["rate"]
            pace["iters"] = max(pace["iters"] - 1, 1)
            pops = 0
            while (filler and pops < 8 and filler[0][3] <= pace["itc"]
                   and (pace["credit"] >= 1.0 or not filler[0][2])):
                if pop_one():
                    pace["credit"] -= 1.0
                pops += 1

        def set_iters(n):
            pace["iters"] = max(n, 1)
            pace["rate"] = _pe_len() / pace["iters"]

        def need(label):
            if label in pending and label not in done:
                while label not in done:
                    if pop_one():
                        pace["credit"] -= 1.0

        def add_chunks(chunks, pe=True, gate=0):
            for item in chunks:
                label, fn = item[0], item[1]
                is_pe = item[2] if len(item) > 2 else pe
                if label:
                    pending.add(label)
                filler.append((label, fn, is_pe, pace["itc"] + gate))
            pace["rate"] = _pe_len() / pace["iters"]

        def qproj_chunks(j):
            # block j of the Q projection: 8 matmul chunks + 1 bias evac
            st = {}

            def mm(kt):
                def go():
                    if kt == 0:
                        st["ps"] = [pp.tile([128, SB], F32, name=f"qps_{j}_{i}", tag="pp")
                                    for i in range(2)]
                        st["x"] = xq_t[j]
                    for dht in range(2):
                        nc.tensor.matmul(st["ps"][dht],
                                         wq_sb[:, kt, dht * 128:(dht + 1) * 128],
                                         st["x"][:, kt, :],
                                         start=(kt == 0), stop=(kt == NKT - 1))
                return go

            def evac():
                for dht in range(2):
                    nc.vector.tensor_scalar_add(qT[:, dht, j * SB:(j + 1) * SB],
                                                st["ps"][dht], bq_sb[:, dht:dht + 1])
            return [(None, mm(kt)) for kt in range(NKT)] + [(("q", j), evac)]

        def kproj_chunks(sb):
            # seq-sliced: two ~427ns chunks per 128-key quarter (8 narrow
            # matmuls each + that slice's bias evac), so chunks for block sb
            # can slide into block sb's own early attention iterations.
            st = {}

            def part(q, dht):
                def go():
                    if q == 0 and dht == 0:
                        st["ps"] = [pp.tile([128, SB], F32, name=f"kps_{sb}_{i}", tag="pp")
                                    for i in range(2)]
                        st["x"] = xk_t[sb]
                    c = slice(q * 128, (q + 1) * 128)
                    for kt in range(NKT):
                        nc.tensor.matmul(st["ps"][dht][:, c],
                                         wk_sb[:, kt, dht * 128:(dht + 1) * 128],
                                         st["x"][:, kt, c],
                                         start=(kt == 0 and q == 0),
                                         stop=(kt == NKT - 1),
                                         skip_group_check=True)
                    nc.vector.tensor_scalar_add(
                        kT[:, dht, sb * SB + q * 128:sb * SB + (q + 1) * 128],
                        st["ps"][dht][:, c], bk_sb[:, dht:dht + 1])
                return go
            return [(("k", sb * 4 + q) if dht == 1 else None, part(q, dht))
                    for q in range(4) for dht in range(2)]

        def vproj_chunks(sb):
            # seq-sliced like K: two ~427ns chunks per 128-key quarter.
            st = {}

            def part(ss, kh):
                def go():
                    if ss == 0 and kh == 0:
                        st["ps"] = [pp.tile([128, SB], F32, name=f"vps_{sb}_{i}", tag="pp")
                                    for i in range(2)]
                        st["x"] = xv_t[sb]
                    half, grp = ss % 2, ss // 2
                    for kt in range(kh * 4, kh * 4 + 4):
                        # two seq-subtiles share one psum bank; only the first
                        # MM in the bank uses start=True
                        nc.tensor.matmul(st["ps"][grp][:, half * DL:(half + 1) * DL],
                                         st["x"][:, kt, ss * 128:(ss + 1) * 128],
                                         wv_sb[:, kt, :],
                                         start=(kt == 0 and half == 0),
                                         stop=(kt == NKT - 1),
                                         skip_group_check=True)
                    if kh == 1:
                        skt = sb * 4 + grp * 2 + half  # == sb*4 + ss
                        nc.vector.tensor_add(
                            v_sb[:, skt, :, 0:DH],
                            st["ps"][grp][:, half * DL:(half + 1) * DL]
                            .rearrange("p (h d) -> p h d", h=HL),
                            bvb.rearrange("p (h d) -> p h d", h=HL))
                return go
            return [(("v", sb * 4 + ss) if kh == 1 else None, part(ss, kh))
                    for ss in range(4) for kh in range(2)]

        def transpose_pair(isq, pr2):
            # Flip aTpre [q, (qt, hi*64+d)] -> aT [hi*64+d, pair, q] with the
            # DMA-engine XBAR transpose (16x128 tiles). One instruction per
            # (pair, query block): 32 tiles, ~450ns of DMA time, zero
            # compute-engine cost.
            nc.sync.dma_start(
                out=aT[:, pr2, isq * SB:(isq + 1) * SB]
                .rearrange("p (t q) -> p t q", t=4),
                in_=aTpre[:, pr2, isq * 4:isq * 4 + 4, :],
                transpose=True)

        def epilogue_norm(isq, pr2, engs=("vector",)):
            # Normalize one pair's query block: reciprocal of the row-sums
            # (landed per-partition in uacc col 64) then per-partition scalar
            # multiplies into aTpre [q, hi*64+d] bf16, alternating engines
            # (DVE + idle Pool) to halve the chain latency. Each half's XBAR
            # transpose fires as soon as its muls land. No PE work.
            st = {}

            def recip():
                rt = misc.tile([128, 4, 2], F32, name=f"ri_{isq}_{pr2}", tag="rinv")
                # NOTE: reciprocal_approx_fast (custom DVE ucode) returns
                # garbage on this axon terminal — standard reciprocal only.
                with nc.allow_low_precision(reason="fp32r rounding of 1/rowsum"):
                    nc.vector.reciprocal(
                        rt, uacc[:, isq * 4:isq * 4 + 4,
                                 pr2 * 2:pr2 * 2 + 2, DH])
                st["r"] = rt

            def mul(qt, hi):
                eng = engs[(qt * 2 + hi) % len(engs)]

                def go():
                    h = pr2 * 2 + hi
                    o = aTpre[:, pr2, isq * 4 + qt, hi * DH:(hi + 1) * DH]
                    i = uacc[:, isq * 4 + qt, h, 0:DH]
                    s = st["r"][:, qt, hi:hi + 1]
                    if eng == "scalar":
                        nc.scalar.activation(o, i, AF.Copy, scale=s)
                    else:
                        getattr(nc, eng).tensor_scalar_mul(o, i, s)
                return go

            out = [(None, recip, False)]
            for qt in range(4):
                for hi in range(2):
                    out.append((None, mul(qt, hi), False))
            out.append((("t", isq, pr2),
                        (lambda: transpose_pair(isq, pr2)), False))
            return out

        cur = {"pt": None, "pt1": None, "pt2": None}

        def guard_mm(op):
            if cur["pt"] is not None:
                # guard: a 1-column matmul reading an already-complete pt
                # ties this chunk to live attention progress, so the Tile
                # scheduler (whose DMA-transpose estimate is optimistic)
                # cannot hoist the outproj ahead of the aT transpose and
                # stall the PE. The real matmul's start=True clears the bank.
                nc.tensor.matmul(op[0:64, 0:1], cur["pt"][:, 0:64],
                                 cur["pt"][:, 0:1],
                                 start=True, stop=True,
                                 skip_group_check=True)

        def epilogue_out(isq, fine=False):
            # outproj: one 128-row tile (~427ns) per mm chunk; evacuation +
            # DMA paired per 256-row group (half on DVE, half on ACT, one
            # DMA) to keep the HWDGE count down.
            # fine=True (the post-attention drain): per-mt evacuation and DMA
            # for lower chain latency, and op tiles alternate between the pp
            # and (now free) p_ut pools so 4 tiles are in flight and the PE
            # never waits on an evacuation.
            q0 = isq * SB
            st = {}

            def mm(mt):
                def go():
                    pool = p_ut if fine and mt % 2 else pp
                    op = pool.tile([128, SB], F32, name=f"op_{isq}_{mt}",
                                   tag="ut" if pool is p_ut else "pp")
                    guard_mm(op)
                    for jt in range(2):
                        nc.tensor.matmul(op, wo_sb[:, jt, mt * 128:(mt + 1) * 128],
                                         aT[:, jt, q0:q0 + SB],
                                         start=(jt == 0), stop=(jt == 1),
                                         skip_group_check=True)
                    st[mt] = op
                return go

            def evac(mg, split_dma=False):
                def go():
                    ops = [st.pop(mg * 2), st.pop(mg * 2 + 1)]
                    ot = outp.tile([128, 2 * SB], BF16, name="ot", tag="ot")
                    # Both evacuations on DVE (the ACT is the bottleneck
                    # engine — exp stream — and must not pay for copies).
                    # Output partials go back bf16 (host accumulates in
                    # fp32): halves the out-DMA drain at the tail.
                    nc.vector.tensor_copy(ot[:, 0:SB], ops[0])
                    if split_dma:
                        # last group: fire each half as soon as its copy
                        # lands so the final DMA is half as long
                        nc.sync.dma_start(out=out4[mg, :, 0, q0:q0 + SB],
                                          in_=ot[:, 0:SB])
                        nc.vector.tensor_copy(ot[:, SB:2 * SB], ops[1])
                        nc.sync.dma_start(out=out4[mg, :, 1, q0:q0 + SB],
                                          in_=ot[:, SB:2 * SB])
                    else:
                        nc.vector.tensor_copy(ot[:, SB:2 * SB], ops[1])
                        nc.sync.dma_start(out=out4[mg, :, :, q0:q0 + SB],
                                          in_=ot.rearrange("p (m s) -> p m s", m=2))
                return go

            out = []
            for mg in range(4):
                out.append((None, mm(mg * 2)))
                out.append((None, mm(mg * 2 + 1)))
                out.append((None, evac(mg, split_dma=(fine and mg == 3))))
            return out

        # One-iteration software pipeline: each iteration's P@V matmuls (and
        # the U evacuation that follows a group's last PV) are deferred and
        # emitted right after the NEXT iteration's exp, so the PE-queue order
        # per exp window is [scores(k+1), filler..., PV(k)]. The PV lands
        # exactly when exp(k) completes and the ACT runs exp back-to-back
        # instead of idling through PV(k) + scores(k+1) each iteration.
        pend = {"q": []}

        def flush_pend():
            for fn in pend["q"]:
                fn()
            pend["q"] = []

        def attention(skt_lo, skt_hi, pr2, isq):
            q0 = isq * SB
            need(("q", isq))
            # one full PSUM bank per hi: start=True zeroes the whole 2KB
            # bank, so only the group's very first matmul into the bank may
            # carry it (the other q-subtiles' slices would be wiped)
            ub = [p_ut.tile([128, 4, 128], F32,
                            name=f"u_{skt_lo}_{pr2}_{isq}_{hi}", tag="ut")
                  for hi in range(2)]
            first, last_ = skt_lo, skt_hi - 1
            for skt in range(skt_lo, skt_hi):
                need(("k", skt))
                stt = p_big.tile([128, 2 * SB], F32, name="stt", tag="big")
                for hi in range(2):
                    od = hi * DH
                    nc.tensor.matmul(stt[:, hi * SB:(hi + 1) * SB],
                                     kT[od:od + DH, pr2, skt * 128:(skt + 1) * 128],
                                     qT[od:od + DH, pr2, q0:q0 + SB],
                                     start=True, stop=True)
                pt = ptp.tile([128, 2 * SB], BF16, name="pt", tag="pt")
                nc.scalar.activation(pt, stt, AF.Exp, scale=SCALE)
                # guard target: the exp two iterations back is complete by
                # the time a pump-popped chunk runs, so guarding on it adds
                # no real wait but still stops deep scheduler hoisting
                cur["pt"], cur["pt1"] = cur.get("pt1"), cur.get("pt2")
                cur["pt2"] = pt
                flush_pend()
                pump()
                need(("v", skt))

                def pv(pt=pt, skt=skt):
                    for hi in range(2):
                        h = pr2 * 2 + hi
                        for qt in range(4):
                            nc.tensor.matmul(ub[hi][:, qt, 0:DH + 1],
                                             pt[:, hi * SB + qt * 128:
                                                hi * SB + (qt + 1) * 128],
                                             v_sb[:, skt, h, :],
                                             start=(skt == first and qt == 0),
                                             stop=(skt == last_),
                                             skip_group_check=True)
                pend["q"].append(pv)

            def evac():
                for hi in range(2):
                    h = pr2 * 2 + hi
                    sl = uacc[:, isq * 4:isq * 4 + 4, h, :]
                    if skt_lo == 0:
                        nc.vector.tensor_copy(sl, ub[hi][:, :, 0:DH + 1])
                    else:
                        nc.vector.tensor_add(sl, sl, ub[hi][:, :, 0:DH + 1])
            pend["q"].append(evac)

        # ---------- prologue: Q/K/V projections for block 0 ----------
        # DMA emission in need-time order: each chunk lands just before the
        # projection matmuls that consume it reach the head of the PE queue.
        # ~14 dummy matmuls on the (memset-only) ones_row run during the
        # initial DMA wait: they start the PE p-state ramp ~2us early so the
        # real projection matmuls run at full clock.
        warm_ps = pp.tile([128, 64], F32, name="warm_ps", tag="pp")
        for _ in range(26):
            nc.tensor.matmul(warm_ps, ones_row, ones_row[:, 0:64],
                             start=True, stop=True, skip_group_check=True)
        xq_t, xk_t, xv_t = {}, {}, {}
        xq_t[0] = xin.tile([128, NKT, SB], BF16, name="x_xqT_0", tag="x")
        xk_t[0] = xin.tile([128, NKT, SB], BF16, name="x_xkT_0", tag="x")
        xv_t[0] = xin.tile([128, NKT, SB], BF16, name="x_xvT_0", tag="x")

        def load_w_part(w_sb, name, kt0, ktn):
            nc.sync.dma_start(
                out=w_sb[:, kt0:kt0 + ktn, :],
                in_=t[name][kt0 * 128:(kt0 + ktn) * 128, :]
                .rearrange("(c p) d -> p c d", p=128))

        def xpart(xt, name, k0, kn):
            nc.sync.dma_start(
                out=xt[:, k0:k0 + kn, :],
                in_=t[name][k0 * 128:(k0 + kn) * 128, 0:SB]
                .rearrange("(c p) s -> p c s", p=128))

        # HWDGE issues one DMA per ~650ns, so the prologue uses few, large
        # transfers in need order; xk goes through the Pool-engine SWDGE
        # path instead, which generates descriptors off the HWDGE queue (it
        # is issued behind the Pool memsets so its transfer slots in after
        # xq on the shared DMA engines).
        # xk's SWDGE descriptor generation must lead the Pool queue so its
        # transfer slots in right after xq on the shared DMA engines
        nc.gpsimd.dma_start(
            out=xk_t[0],
            in_=t["xkT"][:, 0:SB].rearrange("(c p) s -> p c s", p=128))
        xpart(xq_t[0], "xqT", 0, 1)
        load_w_part(wq_sb, "wqT", 0, NKT)
        xpart(xq_t[0], "xqT", 1, 3)
        xpart(xq_t[0], "xqT", 4, 4)
        load_w_half(wk_sb, "wkT", 0)
        load_w_half(wk_sb, "wkT", 1)
        load_biases()
        load_w_half(wv_sb, "wvT", 0)
        load_w_half(wv_sb, "wvT", 1)
        xpart(xv_t[0], "xvT", 0, NKT)
        for _, fn in qproj_chunks(0):
            fn()
        kp0 = kproj_chunks(0)
        for _, fn in kp0[:2]:
            fn()
        # K0 quarters 1-3 and all of V0 ride block 0's first attention
        # iterations (need() pulls them in emission order; kproj's chunks
        # stay contiguous before vproj's so the pp psum ring rotation stays
        # FIFO-safe).
        add_chunks(kp0[2:] + vproj_chunks(0))

        def kv_interleaved(sb):
            # interleave K and V chunks in need order (k_q, v_q alternating)
            ks, vs = kproj_chunks(sb), vproj_chunks(sb)
            out = []
            for q in range(4):
                out += ks[2 * q:2 * q + 2] + vs[2 * q:2 * q + 2]
            return out

        # ---------- main loop ----------
        # Phase A (key blocks 0,1): sb-outer so the K/V projections for the
        # next key block ride the current block's exp bubbles.
        # Phase B (key blocks 2,3): isq-outer with both key blocks merged per
        # (isq, pair) group — U stays PSUM-resident over 8 iterations (one
        # evacuation instead of two) and each query block's epilogue
        # (normalize -> transpose -> output projection) becomes available a
        # quarter-phase earlier, spreading the tail work across the whole
        # phase instead of piling it after the last exp.
        for sb in range(2):
            if sb == 0:
                # everything the rest of sb0 needs, in deadline order
                for j in range(1, NSB):
                    xq_t[j] = load_x("xqT", j)
                xk_t[1] = load_x("xkT", 1)
                xv_t[1] = load_x("xvT", 1)
                nc.sync.dma_start(out=wo_sb,
                                  in_=t["woT"].rearrange("(c p) m -> p c m", p=128))
                for j in range(1, NSB):
                    add_chunks(qproj_chunks(j))
                add_chunks(kv_interleaved(1))
                set_iters(8 * NSB + 8)
            else:
                xk_t[2] = load_x("xkT", 2)
                xv_t[2] = load_x("xvT", 2)
                xk_t[3] = load_x("xkT", 3)
                xv_t[3] = load_x("xvT", 3)
                add_chunks(kv_interleaved(2))
                add_chunks(kv_interleaved(3))
                # paces KV3 into phase B's first iterations (its deadline)
                set_iters(8 * NSB + 16)
            for isq in range(NSB):
                for pr2 in range(PAIRS):
                    attention(sb * 4, sb * 4 + 4, pr2, isq)
        for isq in range(NSB):
            set_iters(4 * NSB)
            for pr2 in range(PAIRS):
                last = (isq == NSB - 1 and pr2 == PAIRS - 1)
                attention(8, 16, pr2, isq)
                if last:
                    # final group: drain the pipelined PV/evac, then emit the
                    # normalize + transpose inline (norm muls split DVE/ACT —
                    # the exp stream is done) so the chain starts without
                    # queueing behind remaining filler
                    flush_pend()
                    for it in epilogue_norm(isq, pr2):
                        it[1]()
                else:
                    add_chunks(epilogue_norm(isq, pr2))
            # gate: the normalize -> transpose chain above needs ~5
            # iterations before aT is ready; popping the outproj earlier
            # stalls the PE on the transpose DMA
            add_chunks(epilogue_out(isq, fine=(isq == NSB - 1)), gate=5)
        while filler:
            pop_one()
        if t.get("dbgA") is not None:
            nc.sync.dma_start(out=t["dbgA"], in_=aT.rearrange("p a s -> p (a s)"))
            nc.sync.dma_start(out=t["dbgU"],
                              in_=uacc.rearrange("p a b c -> p (a b c)"))
            nc.sync.dma_start(out=t["dbgP"],
                              in_=aTpre.rearrange("p a b c -> p (a b c)"))
            nc.sync.dma_start(out=t["dbgQ"],
                              in_=qT.rearrange("p a s -> p (a s)").bitcast(F32))
            nc.sync.dma_start(out=t["dbgK"],
                              in_=kT.rearrange("p a s -> p (a s)").bitcast(F32))
            nc.sync.dma_start(out=t["dbgV"], in_=v_flat.rearrange("p a b -> p (a b)"))


DEBUG_DUMPS = False


def build():
    nc = bacc.Bacc("TRN2", target_bir_lowering=False, debug=False, num_devices=NCORES)
    t = {}
    for name, shape in [("xqT", [D, S]), ("xkT", [D, S]), ("xvT", [D, S]),
                        ("wqT", [D, DL]), ("wkT", [D, DL]), ("wvT", [D, DL]),
                        ("woT", [DL, D])]:
        t[name] = nc.dram_tensor(name, shape, BF16, kind="ExternalInput").ap()
    for name, shape in [("bq", [DL]), ("bk", [DL]), ("bv", [DL])]:
        t[name] = nc.dram_tensor(name, shape, F32, kind="ExternalInput").ap()
    t["outF"] = nc.dram_tensor("outF", [D, S], BF16, kind="ExternalOutput").ap()
    if DEBUG_DUMPS:
        t["dbgA"] = nc.dram_tensor("dbgA", [128, 2 * S], BF16,
                                   kind="ExternalOutput").ap()
        t["dbgU"] = nc.dram_tensor("dbgU", [128, NQT * HL * (DH + 1)], F32,
                                   kind="ExternalOutput").ap()
        t["dbgP"] = nc.dram_tensor("dbgP", [128, PAIRS * NQT * 128], BF16,
                                   kind="ExternalOutput").ap()
        t["dbgQ"] = nc.dram_tensor("dbgQ", [128, 2 * S], F32,
                                   kind="ExternalOutput").ap()
        t["dbgK"] = nc.dram_tensor("dbgK", [128, 2 * S], F32,
                                   kind="ExternalOutput").ap()
        t["dbgV"] = nc.dram_tensor("dbgV", [128, NSKT * HL * (DH + 1)], BF16,
                                   kind="ExternalOutput").ap()
    with tile.TileContext(nc) as tc:
        _emit(tc, nc, t)
    nc.compile()
    return nc


def _bf16(a):
    import ml_dtypes
    return np.ascontiguousarray(np.asarray(a, dtype=np.float32)).astype(ml_dtypes.bfloat16)


def shard(inputs):
    q = np.asarray(inputs["query"], dtype=np.float32)
    k = np.asarray(inputs["key"], dtype=np.float32)
    v = np.asarray(inputs["value"], dtype=np.float32)
    Wq = np.asarray(inputs["Wq"], dtype=np.float32)
    Wk = np.asarray(inputs["Wk"], dtype=np.float32)
    Wv = np.asarray(inputs["Wv"], dtype=np.float32)
    Wo = np.asarray(inputs["Wo"], dtype=np.float32)
    bq = np.asarray(inputs["bq"], dtype=np.float32)
    bk = np.asarray(inputs["bk"], dtype=np.float32)
    bv = np.asarray(inputs["bv"], dtype=np.float32)
    xT = [(_bf16(q[b].T), _bf16(k[b].T), _bf16(v[b].T)) for b in range(B)]
    maps = []
    for c in range(NCORES):
        b, hb = divmod(c, NCORES // B)
        js = slice(hb * DL, (hb + 1) * DL)
        xq, xk, xv = xT[b]
        maps.append({
            "xqT": xq, "xkT": xk, "xvT": xv,
            "wqT": _bf16(Wq[js].T),
            "wkT": _bf16(Wk[js].T),
            "wvT": _bf16(Wv[js].T),
            "woT": _bf16(Wo[:, js].T),
            "bq": np.ascontiguousarray(bq[js]),
            "bk": np.ascontiguousarray(bk[js]),
            "bv": np.ascontiguousarray(bv[js]),
        })
    return maps


def unshard(results, inputs):
    bo = np.asarray(inputs["bo"], dtype=np.float32)
    out = np.empty((B, S, D), np.float32)
    g = NCORES // B
    for b in range(B):
        acc = results[b * g]["outF"].astype(np.float32)
        for i in range(1, g):
            acc += results[b * g + i]["outF"].astype(np.float32)
        out[b] = acc.T + bo
    return out


def kernel(**inputs):
    global LAST_EXEC_NS
    nc = build()
    maps = shard(inputs)
    res = run_bass_kernel_spmd(nc, maps, core_ids=list(range(NCORES)),
                               trace=_TRACE, **_TRACE_KW)
    LAST_EXEC_NS = res.exec_time_ns
    return unshard(res.results, inputs)


# revision 67
# speedup vs baseline: 1.0249x; 1.0249x over previous
"""Multi-head attention Trainium2 kernel (Bass/Tile), 8-core SPMD.

Problem: B=2, S=2048, D=1024, H=16 heads of d=64.
Sharding: core c -> batch c//4, 4 heads starting at 4*(c%4).
Each core computes its heads' Q/K/V projections, attention, and the
partial output projection (transposed); host sums the 4 bf16 partials
per batch and adds bo.

Device-side layout notes:
  - Projection activations live transposed ([feature, seq]) so every
    matmul contraction runs over the partition dim.
  - Scores are computed transposed (S^T[sk, sq]) so that P^T = exp(S^T)
    feeds the P@V matmul directly as the STATIONARY operand.
  - P@V runs with V as the MOVING operand: out tiles are [128 q, 65]
    per (head, q-subtile) with pt [128 k, 128 q] stationary. The PE is
    charged per moving column, so each k-chunk costs 65 cycles instead
    of the 512 a [65, 512]-out layout pays (the old layout used only
    65 of 128 output partitions). V carries an extra ones column so
    the same matmuls emit softmax row-sums for free (col 64).
  - U accumulates in SBUF as [q-part, qtile, head, 65] fp32; row-sums
    land per-partition, so normalization is a plain per-partition
    tensor_scalar multiply on DVE (no partition broadcast needed).
  - The normalized attention is built as aTpre [q, d-of-pair] bf16 and
    flipped to aT [d, q] with the DMA-engine XBAR transpose (16x128
    tiles, off all compute engines); the output projection then runs
    unchanged with aT as the moving operand.
  - x/weights/aT/pt/v/output are bf16 (same PE rate as fp32r at 256+
    moving cols, required for full rate at 65 cols); scores stay fp32.

Schedule: the attention inner loop is ACT-bound (exp of a [128,1024]
score tile is ~1040ns vs ~645ns of score+PV matmuls), so all
projection and epilogue work is chopped into ~430ns chunks and emitted
between the exp and PV of each iteration via a credit-paced filler
queue (labels + need() keep emission order consistent with data flow).
Q proj for block j rides block j-1's iterations; K/V proj for key
block sb+1 ride sb's; normalize (DVE) + transpose (DMA) + output
projection for query block isq ride the last key block's later
iterations. PSUM: 2 banks filler ring + 4 banks score tiles + 2 banks
U accumulators.
"""

import numpy as np

import concourse.bass as bass
import concourse.mybir as mybir
import concourse.tile as tile
from concourse import bacc
from concourse.bass_utils import run_bass_kernel_spmd

F32 = mybir.dt.float32
F32R = mybir.dt.float32r
BF16 = mybir.dt.bfloat16
AF = mybir.ActivationFunctionType

B, S, D = 2, 2048, 1024
H, DH = 16, 64
NCORES = 8
HL = H // (NCORES // B)       # 4 heads per core
DL = HL * DH                  # 256 local projection dims
PAIRS = HL // 2               # 2 head pairs (packed into 128 partitions)
NKT = D // 128                # 8 contraction tiles for projections
SB = 512                      # seq block (matmul moving-dim chunk)
NSB = S // SB                 # 4
NSKT = S // 128               # 16 key-seq tiles
NQT = S // 128                # 16 query subtiles of 128
SCALE = 0.125                 # 1/sqrt(64)
# f32 whose bit pattern is two bf16 1.0s — used to memset a bf16 tile
# to all-ones through its f32 bitcast view (strided/bf16 memsets fail
# walrus ISA checks).
ONES2_BF16 = 1.0019378662109375

LAST_EXEC_NS = None
_TRACE = False
_TRACE_KW = {}


def _bcast_part(ap, parts):
    """View `ap` with the partition dim replaced by a step-0 broadcast."""
    return bass.AP(tensor=ap.tensor, offset=ap.offset, ap=[[0, parts]] + list(ap.ap[1:]))


def _emit(tc, nc, t):
    import contextlib

    ctx = contextlib.ExitStack()
    with ctx:
        const = ctx.enter_context(tc.tile_pool(name="const", bufs=1))
        persist = ctx.enter_context(tc.tile_pool(name="persist", bufs=1))
        # x chunks are one DMA per (tensor, s-block): [128, 8, 512] bf16, 8KB
        # per partition; 6 bufs = one in use + prefetch headroom.
        xin = ctx.enter_context(tc.tile_pool(name="xin", bufs=6))
        ptp = ctx.enter_context(tc.tile_pool(name="ptp", bufs=4))
        outp = ctx.enter_context(tc.tile_pool(name="outp", bufs=4))
        misc = ctx.enter_context(tc.tile_pool(name="misc", bufs=4))

        # ---------- constants & weights ----------
        # Weight DMAs are split in halves and emitted interleaved with the x
        # chunks that pair with them (inside the projection loops below), so
        # the first matmul starts ~2us in instead of waiting ~14us of
        # serialized weight transfers. Weights and x are bf16: same PE rate
        # as fp32r in this regime but half the HBM traffic.
        wq_sb = const.tile([128, NKT, DL], BF16)
        wk_sb = const.tile([128, NKT, DL], BF16)
        wv_sb = const.tile([128, NKT, DL], BF16)
        wo_sb = const.tile([128, 2, D], BF16)
        bq_sb = const.tile([128, 2], F32)
        bk_sb = const.tile([128, 2], F32)
        bvb = const.tile([128, DL], F32)

        def load_biases():
            # bq/bk ride the Pool SWDGE queue (tiny transfers, off the
            # HWDGE/DMA critical path of the prologue); the broadcast bvb
            # stays on HWDGE and is only needed by the V evacuations.
            nc.gpsimd.dma_start(out=bq_sb, in_=t["bq"].rearrange("(t p) -> p t", p=128))
            nc.gpsimd.dma_start(out=bk_sb, in_=t["bk"].rearrange("(t p) -> p t", p=128))
            nc.sync.dma_start(out=bvb,
                              in_=_bcast_part(t["bv"].rearrange("(o d) -> o d", o=1), 128))

        # ---------- persistent activations ----------
        qT = persist.tile([128, PAIRS, S], F32R)   # [dh-in-pair, pair, s]
        kT = persist.tile([128, PAIRS, S], F32R)
        v_flat = persist.tile([128, NSKT, HL * (DH + 1)], BF16)
        # [sk-in-tile, skt, head, d|1]
        v_sb = v_flat.rearrange("p t (h d) -> p t h d", h=HL)
        aT = persist.tile([128, PAIRS, S], BF16)   # normalized attention, transposed
        aTpre = persist.tile([128, PAIRS, NQT, 128], BF16)  # [q, pair, qt, hi*64+d]
        uacc = persist.tile([128, NQT, HL, DH + 1], F32)    # U accumulator (SBUF)
        # Whole-tile memset through the f32 bitcast view: value is the bit
        # pattern of two bf16 1.0s. The V evacuations overwrite everything
        # except the ones column.
        nc.gpsimd.memset(v_flat.bitcast(F32), ONES2_BF16)
        ones_row = persist.tile([1, 128], F32R)
        nc.vector.memset(ones_row.bitcast(F32), 1.0)
        # Warm-up exp so the ACT table set loads at t~0 instead of on the
        # first real exp's critical path.
        warm = persist.tile([1, 1], F32)
        nc.scalar.activation(warm, ones_row.bitcast(F32)[0:1, 0:1], AF.Exp)

        # ---------- shared psum pools ----------
        # 8 banks total: pp 2 (projection/outproj ring), p_big 4 (score
        # tiles), p_ut 2 (U accumulators).
        pp = ctx.enter_context(tc.tile_pool(name="pp", bufs=2, space="PSUM"))
        p_big = ctx.enter_context(tc.tile_pool(name="p_big", bufs=2, space="PSUM"))
        p_ut = ctx.enter_context(tc.tile_pool(name="p_ut", bufs=2, space="PSUM"))

        # out DRAM viewed [mg, p, mi, s]: row (mg*2+mi)*128 + p, col s
        out4 = t["outF"].rearrange("(g m p) s -> g p m s", g=4, m=2)

        def load_w_half(w_sb, name, half):
            nc.sync.dma_start(
                out=w_sb[:, half * 4:(half + 1) * 4, :],
                in_=t[name][half * 512:(half + 1) * 512, :]
                .rearrange("(c p) d -> p c d", p=128))

        def load_x(name, sb, parts=(NKT,)):
            # one DMA per (tensor, s-block); the prologue passes graded part
            # sizes (small first) so the first matmuls start ~2us in.
            xt = xin.tile([128, NKT, SB], BF16, name=f"x_{name}_{sb}", tag="x")
            k0 = 0
            for kpp in parts:
                nc.sync.dma_start(
                    out=xt[:, k0:k0 + kpp, :],
                    in_=t[name][k0 * 128:(k0 + kpp) * 128,
                                sb * SB:(sb + 1) * SB]
                    .rearrange("(c p) s -> p c s", p=128))
                k0 += kpp
            return xt

        # ---------- filler machinery ----------
        # PE work chunks (~400-900ns each) queued up and emitted between the
        # exp and PV of each attention iteration: the attention inner loop is
        # otherwise ACT-bound (exp ~1040ns vs ~645ns of PE work per
        # iteration), so the projections/epilogue ride in the PE bubbles for
        # free. Chunks carry labels; attention() declares the chunks it
        # depends on via need() (emission-order = program order, so a
        # consumer emitted before its producer would silently read stale
        # data).
        filler = []
        done, pending = set(), set()
        pace = {"rate": 0.0, "credit": 0.0, "iters": 1, "itc": 0}

        def _pe_len():
            return sum(1 for it in filler if it[2])

        def pop_one():
            label, fn, is_pe, _gate = filler.pop(0)
            fn()
            if label:
                done.add(label)
            return is_pe

        def pump():
            # credit pacing: spread the PE-bearing chunks evenly over the
            # remaining attention iterations of this key block (an unfilled
            # iteration stalls the PE ~400ns). Chunks with no PE content
            # (normalize/transpose/evacuate) ride along for free. A chunk
            # with a gate > current iteration count blocks the queue head:
            # used to hold the output projection back until its normalize ->
            # transpose chain (emitted the same boundary) has had time to
            # land.
            pace["itc"] += 1
            pace["credit"] += pace["rate"]
            pace["iters"] = max(pace["iters"] - 1, 1)
            pops = 0
            while (filler and pops < 8 and filler[0][3] <= pace["itc"]
                   and (pace["credit"] >= 1.0 or not filler[0][2])):
                if pop_one():
                    pace["credit"] -= 1.0
                pops += 1

        def set_iters(n):
            pace["iters"] = max(n, 1)
            pace["rate"] = _pe_len() / pace["iters"]

        def need(label):
            if label in pending and label not in done:
                while label not in done:
                    if pop_one():
                        pace["credit"] -= 1.0

        def add_chunks(chunks, pe=True, gate=0):
            for item in chunks:
                label, fn = item[0], item[1]
                is_pe = item[2] if len(item) > 2 else pe
                if label:
                    pending.add(label)
                filler.append((label, fn, is_pe, pace["itc"] + gate))
            pace["rate"] = _pe_len() / pace["iters"]

        def qproj_chunks(j):
            # block j of the Q projection: 8 matmul chunks + 1 bias evac
            st = {}

            def mm(kt):
                def go():
                    if kt == 0:
                        st["ps"] = [pp.tile([128, SB], F32, name=f"qps_{j}_{i}", tag="pp")
                                    for i in range(2)]
                        st["x"] = xq_t[j]
                    for dht in range(2):
                        nc.tensor.matmul(st["ps"][dht],
                                         wq_sb[:, kt, dht * 128:(dht + 1) * 128],
                                         st["x"][:, kt, :],
                                         start=(kt == 0), stop=(kt == NKT - 1))
                return go

            def evac():
                for dht in range(2):
                    nc.vector.tensor_scalar_add(qT[:, dht, j * SB:(j + 1) * SB],
                                                st["ps"][dht], bq_sb[:, dht:dht + 1])
            return [(None, mm(kt)) for kt in range(NKT)] + [(("q", j), evac)]

        def kproj_chunks(sb):
            # seq-sliced: two ~427ns chunks per 128-key quarter (8 narrow
            # matmuls each + that slice's bias evac), so chunks for block sb
            # can slide into block sb's own early attention iterations.
            st = {}

            def part(q, dht):
                def go():
                    if q == 0 and dht == 0:
                        st["ps"] = [pp.tile([128, SB], F32, name=f"kps_{sb}_{i}", tag="pp")
                                    for i in range(2)]
                        st["x"] = xk_t[sb]
                    c = slice(q * 128, (q + 1) * 128)
                    for kt in range(NKT):
                        nc.tensor.matmul(st["ps"][dht][:, c],
                                         wk_sb[:, kt, dht * 128:(dht + 1) * 128],
                                         st["x"][:, kt, c],
                                         start=(kt == 0 and q == 0),
                                         stop=(kt == NKT - 1),
                                         skip_group_check=True)
                    nc.vector.tensor_scalar_add(
                        kT[:, dht, sb * SB + q * 128:sb * SB + (q + 1) * 128],
                        st["ps"][dht][:, c], bk_sb[:, dht:dht + 1])
                return go
            return [(("k", sb * 4 + q) if dht == 1 else None, part(q, dht))
                    for q in range(4) for dht in range(2)]

        def vproj_chunks(sb):
            # seq-sliced like K: two ~427ns chunks per 128-key quarter.
            st = {}

            def part(ss, kh):
                def go():
                    if ss == 0 and kh == 0:
                        st["ps"] = [pp.tile([128, SB], F32, name=f"vps_{sb}_{i}", tag="pp")
                                    for i in range(2)]
                        st["x"] = xv_t[sb]
                    half, grp = ss % 2, ss // 2
                    for kt in range(kh * 4, kh * 4 + 4):
                        # two seq-subtiles share one psum bank; only the first
                        # MM in the bank uses start=True
                        nc.tensor.matmul(st["ps"][grp][:, half * DL:(half + 1) * DL],
                                         st["x"][:, kt, ss * 128:(ss + 1) * 128],
                                         wv_sb[:, kt, :],
                                         start=(kt == 0 and half == 0),
                                         stop=(kt == NKT - 1),
                                         skip_group_check=True)
                    if kh == 1:
                        skt = sb * 4 + grp * 2 + half  # == sb*4 + ss
                        nc.vector.tensor_add(
                            v_sb[:, skt, :, 0:DH],
                            st["ps"][grp][:, half * DL:(half + 1) * DL]
                            .rearrange("p (h d) -> p h d", h=HL),
                            bvb.rearrange("p (h d) -> p h d", h=HL))
                return go
            return [(("v", sb * 4 + ss) if kh == 1 else None, part(ss, kh))
                    for ss in range(4) for kh in range(2)]

        def transpose_pair(isq, pr2):
            # Flip aTpre [q, (qt, hi*64+d)] -> aT [hi*64+d, pair, q] with the
            # DMA-engine XBAR transpose (16x128 tiles). One instruction per
            # (pair, query block): 32 tiles, ~450ns of DMA time, zero
            # compute-engine cost.
            nc.sync.dma_start(
                out=aT[:, pr2, isq * SB:(isq + 1) * SB]
                .rearrange("p (t q) -> p t q", t=4),
                in_=aTpre[:, pr2, isq * 4:isq * 4 + 4, :],
                transpose=True)

        def epilogue_norm(isq, pr2, engs=("vector",)):
            # Normalize one pair's query block: reciprocal of the row-sums
            # (landed per-partition in uacc col 64) then per-partition scalar
            # multiplies into aTpre [q, hi*64+d] bf16, alternating engines
            # (DVE + idle Pool) to halve the chain latency. Each half's XBAR
            # transpose fires as soon as its muls land. No PE work.
            st = {}

            def recip():
                rt = misc.tile([128, 4, 2], F32, name=f"ri_{isq}_{pr2}", tag="rinv")
                # NOTE: reciprocal_approx_fast (custom DVE ucode) returns
                # garbage on this axon terminal — standard reciprocal only.
                with nc.allow_low_precision(reason="fp32r rounding of 1/rowsum"):
                    nc.vector.reciprocal(
                        rt, uacc[:, isq * 4:isq * 4 + 4,
                                 pr2 * 2:pr2 * 2 + 2, DH])
                st["r"] = rt

            def mul(qt, hi):
                eng = engs[(qt * 2 + hi) % len(engs)]

                def go():
                    h = pr2 * 2 + hi
                    o = aTpre[:, pr2, isq * 4 + qt, hi * DH:(hi + 1) * DH]
                    i = uacc[:, isq * 4 + qt, h, 0:DH]
                    s = st["r"][:, qt, hi:hi + 1]
                    if eng == "scalar":
                        nc.scalar.activation(o, i, AF.Copy, scale=s)
                    else:
                        getattr(nc, eng).tensor_scalar_mul(o, i, s)
                return go

            out = [(None, recip, False)]
            for qt in range(4):
                for hi in range(2):
                    out.append((None, mul(qt, hi), False))
            out.append((("t", isq, pr2),
                        (lambda: transpose_pair(isq, pr2)), False))
            return out

        cur = {"pt": None, "pt1": None, "pt2": None}

        def guard_mm(op):
            if cur["pt"] is not None:
                # guard: a 1-column matmul reading an already-complete pt
                # ties this chunk to live attention progress, so the Tile
                # scheduler (whose DMA-transpose estimate is optimistic)
                # cannot hoist the outproj ahead of the aT transpose and
                # stall the PE. The real matmul's start=True clears the bank.
                nc.tensor.matmul(op[0:64, 0:1], cur["pt"][:, 0:64],
                                 cur["pt"][:, 0:1],
                                 start=True, stop=True,
                                 skip_group_check=True)

        def epilogue_out(isq, fine=False):
            # outproj: one 128-row tile (~427ns) per mm chunk; evacuation +
            # DMA paired per 256-row group (half on DVE, half on ACT, one
            # DMA) to keep the HWDGE count down.
            # fine=True (the post-attention drain): per-mt evacuation and DMA
            # for lower chain latency, and op tiles alternate between the pp
            # and (now free) p_ut pools so 4 tiles are in flight and the PE
            # never waits on an evacuation.
            q0 = isq * SB
            st = {}

            def mm(mt):
                def go():
                    pool = p_ut if fine and mt % 2 else pp
                    op = pool.tile([128, SB], F32, name=f"op_{isq}_{mt}",
                                   tag="ut" if pool is p_ut else "pp")
                    guard_mm(op)
                    for jt in range(2):
                        nc.tensor.matmul(op, wo_sb[:, jt, mt * 128:(mt + 1) * 128],
                                         aT[:, jt, q0:q0 + SB],
                                         start=(jt == 0), stop=(jt == 1),
                                         skip_group_check=True)
                    st[mt] = op
                return go

            def evac(mg, split_dma=False):
                def go():
                    ops = [st.pop(mg * 2), st.pop(mg * 2 + 1)]
                    ot = outp.tile([128, 2 * SB], BF16, name="ot", tag="ot")
                    # Both evacuations on DVE (the ACT is the bottleneck
                    # engine — exp stream — and must not pay for copies).
                    # Output partials go back bf16 (host accumulates in
                    # fp32): halves the out-DMA drain at the tail.
                    nc.vector.tensor_copy(ot[:, 0:SB], ops[0])
                    if split_dma:
                        # last group: fire each half as soon as its copy
                        # lands so the final DMA is half as long; the second
                        # copy rides the (idle at the tail) ACT engine
                        nc.sync.dma_start(out=out4[mg, :, 0, q0:q0 + SB],
                                          in_=ot[:, 0:SB])
                        nc.scalar.copy(ot[:, SB:2 * SB], ops[1])
                        nc.sync.dma_start(out=out4[mg, :, 1, q0:q0 + SB],
                                          in_=ot[:, SB:2 * SB])
                    elif fine:
                        nc.scalar.copy(ot[:, SB:2 * SB], ops[1])
                        nc.sync.dma_start(out=out4[mg, :, :, q0:q0 + SB],
                                          in_=ot.rearrange("p (m s) -> p m s", m=2))
                    else:
                        nc.vector.tensor_copy(ot[:, SB:2 * SB], ops[1])
                        nc.sync.dma_start(out=out4[mg, :, :, q0:q0 + SB],
                                          in_=ot.rearrange("p (m s) -> p m s", m=2))
                return go

            out = []
            for mg in range(4):
                out.append((None, mm(mg * 2)))
                out.append((None, mm(mg * 2 + 1)))
                out.append((None, evac(mg, split_dma=(fine and mg == 3))))
            return out

        # One-iteration software pipeline: each iteration's P@V matmuls (and
        # the U evacuation that follows a group's last PV) are deferred and
        # emitted right after the NEXT iteration's exp, so the PE-queue order
        # per exp window is [scores(k+1), filler..., PV(k)]. The PV lands
        # exactly when exp(k) completes and the ACT runs exp back-to-back
        # instead of idling through PV(k) + scores(k+1) each iteration.
        pend = {"q": []}

        def flush_pend():
            for fn in pend["q"]:
                fn()
            pend["q"] = []

        def attention(skt_lo, skt_hi, pr2, isq):
            q0 = isq * SB
            need(("q", isq))
            # one full PSUM bank per hi: start=True zeroes the whole 2KB
            # bank, so only the group's very first matmul into the bank may
            # carry it (the other q-subtiles' slices would be wiped)
            ub = [p_ut.tile([128, 4, 128], F32,
                            name=f"u_{skt_lo}_{pr2}_{isq}_{hi}", tag="ut")
                  for hi in range(2)]
            first, last_ = skt_lo, skt_hi - 1
            for skt in range(skt_lo, skt_hi):
                need(("k", skt))
                stt = p_big.tile([128, 2 * SB], F32, name="stt", tag="big")
                for hi in range(2):
                    od = hi * DH
                    nc.tensor.matmul(stt[:, hi * SB:(hi + 1) * SB],
                                     kT[od:od + DH, pr2, skt * 128:(skt + 1) * 128],
                                     qT[od:od + DH, pr2, q0:q0 + SB],
                                     start=True, stop=True)
                pt = ptp.tile([128, 2 * SB], BF16, name="pt", tag="pt")
                nc.scalar.activation(pt, stt, AF.Exp, scale=SCALE)
                # guard target: the exp two iterations back is complete by
                # the time a pump-popped chunk runs, so guarding on it adds
                # no real wait but still stops deep scheduler hoisting
                cur["pt"], cur["pt1"] = cur.get("pt1"), cur.get("pt2")
                cur["pt2"] = pt
                flush_pend()
                pump()
                need(("v", skt))

                def pv(pt=pt, skt=skt):
                    for hi in range(2):
                        h = pr2 * 2 + hi
                        for qt in range(4):
                            nc.tensor.matmul(ub[hi][:, qt, 0:DH + 1],
                                             pt[:, hi * SB + qt * 128:
                                                hi * SB + (qt + 1) * 128],
                                             v_sb[:, skt, h, :],
                                             start=(skt == first and qt == 0),
                                             stop=(skt == last_),
                                             skip_group_check=True)
                pend["q"].append(pv)

            def evac():
                for hi in range(2):
                    h = pr2 * 2 + hi
                    sl = uacc[:, isq * 4:isq * 4 + 4, h, :]
                    if skt_lo == 0:
                        nc.vector.tensor_copy(sl, ub[hi][:, :, 0:DH + 1])
                    else:
                        nc.vector.tensor_add(sl, sl, ub[hi][:, :, 0:DH + 1])
            pend["q"].append(evac)

        # ---------- prologue: Q/K/V projections for block 0 ----------
        # DMA emission in need-time order: each chunk lands just before the
        # projection matmuls that consume it reach the head of the PE queue.
        # ~14 dummy matmuls on the (memset-only) ones_row run during the
        # initial DMA wait: they start the PE p-state ramp ~2us early so the
        # real projection matmuls run at full clock.
        warm_ps = pp.tile([128, 64], F32, name="warm_ps", tag="pp")
        for _ in range(26):
            nc.tensor.matmul(warm_ps, ones_row, ones_row[:, 0:64],
                             start=True, stop=True, skip_group_check=True)
        xq_t, xk_t, xv_t = {}, {}, {}
        xq_t[0] = xin.tile([128, NKT, SB], BF16, name="x_xqT_0", tag="x")
        xk_t[0] = xin.tile([128, NKT, SB], BF16, name="x_xkT_0", tag="x")
        xv_t[0] = xin.tile([128, NKT, SB], BF16, name="x_xvT_0", tag="x")

        def load_w_part(w_sb, name, kt0, ktn):
            nc.sync.dma_start(
                out=w_sb[:, kt0:kt0 + ktn, :],
                in_=t[name][kt0 * 128:(kt0 + ktn) * 128, :]
                .rearrange("(c p) d -> p c d", p=128))

        def xpart(xt, name, k0, kn):
            nc.sync.dma_start(
                out=xt[:, k0:k0 + kn, :],
                in_=t[name][k0 * 128:(k0 + kn) * 128, 0:SB]
                .rearrange("(c p) s -> p c s", p=128))

        # HWDGE issues one DMA per ~650ns, so the prologue uses few, large
        # transfers in need order; xk goes through the Pool-engine SWDGE
        # path instead, which generates descriptors off the HWDGE queue (it
        # is issued behind the Pool memsets so its transfer slots in after
        # xq on the shared DMA engines).
        # xk's SWDGE descriptor generation must lead the Pool queue so its
        # transfer slots in right after xq on the shared DMA engines; xq's
        # remainder goes as ONE part so it enters the DMA FIFO before xk's
        # generation completes
        nc.gpsimd.dma_start(
            out=xk_t[0],
            in_=t["xkT"][:, 0:SB].rearrange("(c p) s -> p c s", p=128))
        xpart(xq_t[0], "xqT", 0, 1)
        load_w_part(wq_sb, "wqT", 0, NKT)
        xpart(xq_t[0], "xqT", 1, NKT - 1)
        load_w_half(wk_sb, "wkT", 0)
        load_w_half(wk_sb, "wkT", 1)
        load_biases()
        load_w_half(wv_sb, "wvT", 0)
        load_w_half(wv_sb, "wvT", 1)
        xpart(xv_t[0], "xvT", 0, NKT)
        for _, fn in qproj_chunks(0):
            fn()
        kp0 = kproj_chunks(0)
        for _, fn in kp0[:2]:
            fn()
        # K0 quarters 1-3 and all of V0 ride block 0's first attention
        # iterations (need() pulls them in emission order; kproj's chunks
        # stay contiguous before vproj's so the pp psum ring rotation stays
        # FIFO-safe).
        add_chunks(kp0[2:] + vproj_chunks(0))

        def kv_interleaved(sb):
            # interleave K and V chunks in need order (k_q, v_q alternating)
            ks, vs = kproj_chunks(sb), vproj_chunks(sb)
            out = []
            for q in range(4):
                out += ks[2 * q:2 * q + 2] + vs[2 * q:2 * q + 2]
            return out

        # ---------- main loop ----------
        # Phase A (key blocks 0,1): sb-outer so the K/V projections for the
        # next key block ride the current block's exp bubbles.
        # Phase B (key blocks 2,3): isq-outer with both key blocks merged per
        # (isq, pair) group — U stays PSUM-resident over 8 iterations (one
        # evacuation instead of two) and each query block's epilogue
        # (normalize -> transpose -> output projection) becomes available a
        # quarter-phase earlier, spreading the tail work across the whole
        # phase instead of piling it after the last exp.
        for sb in range(2):
            if sb == 0:
                # everything the rest of sb0 needs, in deadline order
                for j in range(1, NSB):
                    xq_t[j] = load_x("xqT", j)
                xk_t[1] = load_x("xkT", 1)
                xv_t[1] = load_x("xvT", 1)
                nc.sync.dma_start(out=wo_sb,
                                  in_=t["woT"].rearrange("(c p) m -> p c m", p=128))
                for j in range(1, NSB):
                    add_chunks(qproj_chunks(j))
                add_chunks(kv_interleaved(1))
                set_iters(8 * NSB + 8)
            else:
                xk_t[2] = load_x("xkT", 2)
                xv_t[2] = load_x("xvT", 2)
                xk_t[3] = load_x("xkT", 3)
                xv_t[3] = load_x("xvT", 3)
                add_chunks(kv_interleaved(2))
                add_chunks(kv_interleaved(3))
                # paces KV3 into phase B's first iterations (its deadline)
                set_iters(8 * NSB + 16)
            for isq in range(NSB):
                for pr2 in range(PAIRS):
                    attention(sb * 4, sb * 4 + 4, pr2, isq)
        for isq in range(NSB):
            set_iters(4 * NSB)
            for pr2 in range(PAIRS):
                last = (isq == NSB - 1 and pr2 == PAIRS - 1)
                attention(8, 16, pr2, isq)
                if last:
                    # final group: drain the pipelined PV/evac, then emit the
                    # normalize + transpose inline (norm muls split DVE/ACT —
                    # the exp stream is done) so the chain starts without
                    # queueing behind remaining filler
                    flush_pend()
                    for it in epilogue_norm(isq, pr2):
                        it[1]()
                    # keep-warm dummies through the final transpose wait:
                    # reading the LAST pt pins them after the final exp (the
                    # scheduler can't hoist them into earlier bubbles), and
                    # they stop the PE clock from dropping to the cold
                    # p-state before the last outproj matmuls
                    warm2 = pp.tile([128, 128], F32, name="warm2", tag="pp")
                    for _ in range(82):
                        nc.tensor.matmul(warm2[0:32, :],
                                         cur["pt2"][:, 0:32],
                                         cur["pt2"][:, 0:128],
                                         start=True, stop=True,
                                         skip_group_check=True)
                else:
                    add_chunks(epilogue_norm(isq, pr2))
            # gate: the normalize -> transpose chain above needs ~5
            # iterations before aT is ready; popping the outproj earlier
            # stalls the PE on the transpose DMA
            add_chunks(epilogue_out(isq, fine=(isq == NSB - 1)), gate=7)
        while filler:
            pop_one()
        if t.get("dbgA") is not None:
            nc.sync.dma_start(out=t["dbgA"], in_=aT.rearrange("p a s -> p (a s)"))
            nc.sync.dma_start(out=t["dbgU"],
                              in_=uacc.rearrange("p a b c -> p (a b c)"))
            nc.sync.dma_start(out=t["dbgP"],
                              in_=aTpre.rearrange("p a b c -> p (a b c)"))
            nc.sync.dma_start(out=t["dbgQ"],
                              in_=qT.rearrange("p a s -> p (a s)").bitcast(F32))
            nc.sync.dma_start(out=t["dbgK"],
                              in_=kT.rearrange("p a s -> p (a s)").bitcast(F32))
            nc.sync.dma_start(out=t["dbgV"], in_=v_flat.rearrange("p a b -> p (a b)"))


DEBUG_DUMPS = False


def build():
    nc = bacc.Bacc("TRN2", target_bir_lowering=False, debug=False, num_devices=NCORES)
    t = {}
    for name, shape in [("xqT", [D, S]), ("xkT", [D, S]), ("xvT", [D, S]),
                        ("wqT", [D, DL]), ("wkT", [D, DL]), ("wvT", [D, DL]),
                        ("woT", [DL, D])]:
        t[name] = nc.dram_tensor(name, shape, BF16, kind="ExternalInput").ap()
    for name, shape in [("bq", [DL]), ("bk", [DL]), ("bv", [DL])]:
        t[name] = nc.dram_tensor(name, shape, F32, kind="ExternalInput").ap()
    t["outF"] = nc.dram_tensor("outF", [D, S], BF16, kind="ExternalOutput").ap()
    if DEBUG_DUMPS:
        t["dbgA"] = nc.dram_tensor("dbgA", [128, 2 * S], BF16,
                                   kind="ExternalOutput").ap()
        t["dbgU"] = nc.dram_tensor("dbgU", [128, NQT * HL * (DH + 1)], F32,
                                   kind="ExternalOutput").ap()
        t["dbgP"] = nc.dram_tensor("dbgP", [128, PAIRS * NQT * 128], BF16,
                                   kind="ExternalOutput").ap()
        t["dbgQ"] = nc.dram_tensor("dbgQ", [128, 2 * S], F32,
                                   kind="ExternalOutput").ap()
        t["dbgK"] = nc.dram_tensor("dbgK", [128, 2 * S], F32,
                                   kind="ExternalOutput").ap()
        t["dbgV"] = nc.dram_tensor("dbgV", [128, NSKT * HL * (DH + 1)], BF16,
                                   kind="ExternalOutput").ap()
    with tile.TileContext(nc) as tc:
        _emit(tc, nc, t)
    nc.compile()
    return nc


def _bf16(a):
    import ml_dtypes
    return np.ascontiguousarray(np.asarray(a, dtype=np.float32)).astype(ml_dtypes.bfloat16)


def shard(inputs):
    q = np.asarray(inputs["query"], dtype=np.float32)
    k = np.asarray(inputs["key"], dtype=np.float32)
    v = np.asarray(inputs["value"], dtype=np.float32)
    Wq = np.asarray(inputs["Wq"], dtype=np.float32)
    Wk = np.asarray(inputs["Wk"], dtype=np.float32)
    Wv = np.asarray(inputs["Wv"], dtype=np.float32)
    Wo = np.asarray(inputs["Wo"], dtype=np.float32)
    bq = np.asarray(inputs["bq"], dtype=np.float32)
    bk = np.asarray(inputs["bk"], dtype=np.float32)
    bv = np.asarray(inputs["bv"], dtype=np.float32)
    xT = [(_bf16(q[b].T), _bf16(k[b].T), _bf16(v[b].T)) for b in range(B)]
    maps = []
    for c in range(NCORES):
        b, hb = divmod(c, NCORES // B)
        js = slice(hb * DL, (hb + 1) * DL)
        xq, xk, xv = xT[b]
        maps.append({
            "xqT": xq, "xkT": xk, "xvT": xv,
            "wqT": _bf16(Wq[js].T),
            "wkT": _bf16(Wk[js].T),
            "wvT": _bf16(Wv[js].T),
            "woT": _bf16(Wo[:, js].T),
            "bq": np.ascontiguousarray(bq[js]),
            "bk": np.ascontiguousarray(bk[js]),
            "bv": np.ascontiguousarray(bv[js]),
        })
    return maps


def unshard(results, inputs):
    bo = np.asarray(inputs["bo"], dtype=np.float32)
    out = np.empty((B, S, D), np.float32)
    g = NCORES // B
    for b in range(B):
        acc = results[b * g]["outF"].astype(np.float32)
        for i in range(1, g):
            acc += results[b * g + i]["outF"].astype(np.float32)
        out[b] = acc.T + bo
    return out


def kernel(**inputs):
    global LAST_EXEC_NS
    nc = build()
    maps = shard(inputs)
    res = run_bass_kernel_spmd(nc, maps, core_ids=list(range(NCORES)),
                               trace=_TRACE, **_TRACE_KW)
    LAST_EXEC_NS = res.exec_time_ns
    return unshard(res.results, inputs)
